# revision 22
# baseline (speedup 1.0000x reference)
"""Trainium2 Bass kernel for nn_DayEncoder (BiMamba day encoder).

kernel(**inputs) takes the FULL unsharded inputs (numpy fp32) and returns
(day_embed (16,512) fp32, episode_ctx (16,1020,512) fp32), matching
reference.reference(**inputs).

Sharding: data-parallel over batch B=16 across 8 NeuronCores (2 samples per
core), weights replicated; one SPMD Bass program, host gathers outputs.

On-chip layout: activations [128-partition(d), dtile(4), token(2048)], token
= b*1024 + t. Weights stationary as pre-transposed 128x128 lhsT blocks
(prepared host-side, bf16). The selective scan runs per state-channel n as a
DVE tensor_tensor_scan along the free dim over all 4 dtiles at once (dA
zeroed at each block start resets the recurrence); uB/hC elementwise in bf16
with gpsimd partition-broadcast B/C rows; y accumulated over n into PSUM via
identity matmuls on the tensor engine (exact fp32), one sample at a time to
fit PSUM. Sequence reversal for the backward direction uses paired PE
transposes (plain + anti-diagonal).
"""
import sys

sys.path.insert(0, "/opt/trn_rl_repo")

import numpy as np
import ml_dtypes

BF = ml_dtypes.bfloat16

D_MODEL = 512
D_STATE = 64
N_LAYERS = 12
D_CONV = 4
N_SUM = 4
D_INNER = 512
DT_RANK = 32
B = 16
N_EP = 1020
L = N_SUM + N_EP          # 1024
NB = 2
TOK = NB * L              # 2048
NDT = 4
NCORES = 8
PAD = D_CONV - 1
LP = L + PAD


def _pack_rev_idx(n):
    idx = (n - 1 - np.arange(n)).astype(np.int16)
    nf = n // 32
    arr = np.zeros((16, 2 * nf), np.int16)
    for j in range(n):
        i, k = divmod(j, 32)
        arr[k % 16, 2 * i + k // 16] = idx[j]
    return np.tile(arr, (8, 1))


def _wblocks(wT, kparts, mparts):
    K, M = wT.shape
    out = np.zeros((kparts, 128, mparts, 128), BF)
    for kb in range(kparts):
        for mb in range(mparts):
            blk = wT[kb * 128:(kb + 1) * 128, mb * 128:(mb + 1) * 128]
            out[kb, :blk.shape[0], mb, :blk.shape[1]] = blk.astype(BF)
    return out


def _prep_host(inputs):
    inp = {k: np.asarray(v, dtype=np.float32) for k, v in inputs.items()}
    shared = {}
    shared["fW1"] = _wblocks(inp["fusion_W1"].T, 4, 4)
    shared["fb1"] = np.ascontiguousarray(inp["fusion_b1"].reshape(4, 128).T)
    shared["fW2"] = _wblocks(inp["fusion_W2"].T, 4, 4)
    shared["fb2"] = np.ascontiguousarray(inp["fusion_b2"].reshape(4, 128).T)
    WiT = np.zeros((24, 4, 128, 8, 128), BF)
    WxT = np.zeros((24, 4, 128, 2, 128), BF)
    WdtT = np.zeros((24, 32, 4, 128), BF)
    WoT = np.zeros((24, 4, 128, 4, 128), BF)
    smallw = np.zeros((24, 128, 4, 8), np.float32)  # [.., dt, {w0..w3,cb,dtb,dsk,-}]
    Aneg = np.zeros((24, 128, 4, D_STATE), np.float32)
    for l in range(N_LAYERS):
        for r in range(2):
            lr = 2 * l + r
            WiT[lr] = _wblocks(inp["in_proj_W"][l, r].T, 4, 8)
            WxT[lr] = _wblocks(
                np.pad(inp["x_proj_W"][l, r], ((0, 96), (0, 0))).T, 4, 2)
            WdtT[lr] = inp["dt_proj_W"][l, r].T.reshape(32, 4, 128).astype(BF)
            WoT[lr] = _wblocks(inp["out_proj_W"][l, r].T, 4, 4)
            smallw[lr, :, :, 0:4] = inp["conv_w"][l, r].reshape(
                4, 128, D_CONV).transpose(1, 0, 2)
            smallw[lr, :, :, 4] = inp["conv_b"][l, r].reshape(4, 128).T
            smallw[lr, :, :, 5] = inp["dt_proj_b"][l, r].reshape(4, 128).T
            smallw[lr, :, :, 6] = inp["D_skip"][l, r].reshape(4, 128).T
            Aneg[lr] = (-np.exp(inp["A_log"][l, r])).reshape(
                4, 128, D_STATE).transpose(1, 0, 2)
    shared.update(WiT=WiT, WxT=WxT, WdtT=WdtT, WoT=WoT, smallw=smallw,
                  Aneg=Aneg)
    a_flat = Aneg.transpose(0, 2, 1, 3).reshape(24, 512, D_STATE)
    a0 = a_flat[:, 0, :]
    shared["_A_uniform"] = bool(np.all(
        np.abs(a_flat - a0[:, None, :]) <= 1e-6 * np.abs(a0[:, None, :]) + 1e-12))
    shared["_A_vals"] = a0.astype(np.float64)

    # layer norms + final norm: per-partition scalars [13, 128, 2, 4] fp32
    lngb = np.zeros((13, 2, 4, 128), np.float32)
    for l in range(N_LAYERS):
        lngb[l, 0] = inp["ln_g"][l].reshape(4, 128)
        lngb[l, 1] = inp["ln_b"][l].reshape(4, 128)
    lngb[12, 0] = inp["norm_g"].reshape(4, 128)
    lngb[12, 1] = inp["norm_b"].reshape(4, 128)
    shared["lngb"] = np.ascontiguousarray(
        lngb.transpose(0, 3, 1, 2).reshape(13, 128, 8))
    shared["attnW"] = np.ascontiguousarray(
        inp["attn_W"].reshape(4, 128).T).astype(BF)      # [128, 4]
    shared["_attnb"] = float(inp["attn_b"][0])
    shared["sumT"] = np.ascontiguousarray(
        inp["summary_tokens"][0].T.reshape(4, 128, N_SUM).transpose(1, 0, 2))
    shared["revidx"] = _pack_rev_idx(1024).astype(np.int16)
    shared["Jmat"] = np.ascontiguousarray(np.eye(128, dtype=np.float32)[:, ::-1]).astype(BF)

    feat = np.concatenate([inp["episode_waveform"], inp["episode_rhythm"]], -1)
    per_core = []
    for c in range(NCORES):
        fc = feat[2 * c:2 * c + 2].reshape(2 * N_EP, D_MODEL).T  # [512, 2040]
        per_core.append({"featT": np.ascontiguousarray(
            fc.reshape(4, 128, 2 * N_EP).transpose(1, 0, 2)).astype(BF)})
    return shared, per_core


_BUILD_CACHE = {}


def _build_program(A_vals, A_uniform, attnb):
    import concourse.bass as bass
    import concourse.bacc as bacc_mod
    import concourse.tile as tile
    import concourse.tile_utils as tile_utils
    from concourse import mybir
    from concourse.masks import make_identity
    from contextlib import ExitStack

    try:
        tile_utils.config.max_sbuf_usage = 206 * 2 ** 10
    except Exception:
        pass

    fp32 = mybir.dt.float32
    bf16 = mybir.dt.bfloat16
    AL = mybir.AluOpType
    AF = mybir.ActivationFunctionType

    nc = bacc_mod.Bacc()

    dI, dO = "ExternalInput", "ExternalOutput"
    d_featT = nc.dram_tensor("featT", [128, 4, 2 * N_EP], bf16, kind=dI)
    d_sumT = nc.dram_tensor("sumT", [128, 4, N_SUM], fp32, kind=dI)
    d_fW1 = nc.dram_tensor("fW1", [4, 128, 4, 128], bf16, kind=dI)
    d_fb1 = nc.dram_tensor("fb1", [128, 4], fp32, kind=dI)
    d_fW2 = nc.dram_tensor("fW2", [4, 128, 4, 128], bf16, kind=dI)
    d_fb2 = nc.dram_tensor("fb2", [128, 4], fp32, kind=dI)
    d_WiT = nc.dram_tensor("WiT", [24, 4, 128, 8, 128], bf16, kind=dI)
    d_WxT = nc.dram_tensor("WxT", [24, 4, 128, 2, 128], bf16, kind=dI)
    d_WdtT = nc.dram_tensor("WdtT", [24, 32, 4, 128], bf16, kind=dI)
    d_WoT = nc.dram_tensor("WoT", [24, 4, 128, 4, 128], bf16, kind=dI)
    d_smallw = nc.dram_tensor("smallw", [24, 128, 4, 8], fp32, kind=dI)
    d_Aneg = nc.dram_tensor("Aneg", [24, 128, 4, D_STATE], fp32, kind=dI)
    d_lngb = nc.dram_tensor("lngb", [13, 128, 8], fp32, kind=dI)
    d_attnW = nc.dram_tensor("attnW", [128, 4], bf16, kind=dI)
    d_revidx = nc.dram_tensor("revidx", [128, 64], mybir.dt.int16, kind=dI)
    d_J = nc.dram_tensor("Jmat", [128, 128], bf16, kind=dI)
    o_day = nc.dram_tensor("day", [NB, D_MODEL], fp32, kind=dO)
    o_ctx = nc.dram_tensor("ctx", [NB, N_EP, D_MODEL], fp32, kind=dO)

    ctx = ExitStack()
    with tile.TileContext(nc) as tc, ctx, \
            nc.allow_low_precision(reason="bf16 pipeline validated vs fp32 reference (3.6e-3)"):
        const = ctx.enter_context(tc.tile_pool(name="const", bufs=1))
        persist = ctx.enter_context(tc.tile_pool(name="persist", bufs=1))
        wpool = ctx.enter_context(tc.tile_pool(name="wpool", bufs=1))
        scan_a = ctx.enter_context(tc.tile_pool(name="scan_a", bufs=1))
        uB2 = ctx.enter_context(tc.tile_pool(name="uB2", bufs=1))
        mov2 = ctx.enter_context(tc.tile_pool(name="mov2", bufs=1))
        bcast = ctx.enter_context(tc.tile_pool(name="bcast", bufs=2))
        stgp = ctx.enter_context(tc.tile_pool(name="stgp", bufs=1))
        small = ctx.enter_context(tc.tile_pool(name="small", bufs=1))
        ppool = ctx.enter_context(tc.tile_pool(name="ppool", bufs=2, space="PSUM"))

        # ---------------- constants ----------------
        cbf = const.tile([128, 257], bf16)
        ident16 = cbf[:, 0:128]
        J16 = cbf[:, 128:256]
        ones_b = cbf[:, 256:257]
        make_identity(nc, ident16)
        nc.gpsimd.dma_start(out=J16, in_=d_J[:])
        nc.vector.memset(ones_b, 1.0)
        cf32 = const.tile([128, 4], fp32)
        ones_f = cf32[:, 0:1]
        eps_sb = cf32[0:1, 1:2]
        nc.vector.memset(ones_f, 1.0)
        nc.vector.memset(cf32[:, 1:2], 1e-5)

        attnW_sb = const.tile([128, 4], bf16)
        nc.gpsimd.dma_start(out=attnW_sb, in_=d_attnW[:])
        revidx_sb = const.tile([128, 64], mybir.dt.int16)
        nc.gpsimd.dma_start(out=revidx_sb, in_=d_revidx[:])


        # ---------------- persistent activations ----------------
        x = persist.tile([128, NDT, TOK], fp32)
        hb16 = persist.tile([128, NDT, TOK], bf16)
        zg = persist.tile([128, NDT, TOK], bf16)
        xc = persist.tile([128, NDT, TOK], bf16)
        dtt = persist.tile([128, NDT, TOK], bf16)
        ut = persist.tile([128, NDT, TOK], bf16)
        # packed projection outputs: rows 0-31 dtr, 32-95 Bm, 96-127 Cm[0:32]
        proj = persist.tile([128, TOK], bf16)
        cmhi = persist.tile([32, TOK], bf16)  # Cm[32:64]
        s1m = persist.tile([1, TOK], bf16)    # s1 -> mean (in place)
        s2v = persist.tile([1, TOK], bf16)    # s2 -> ex2 -> var -> rstd
        m2t = persist.tile([1, TOK], bf16)
        wlogt = persist.tile([1, TOK], bf16)
        wsmt = persist.tile([1, TOK], bf16)
        mxT = persist.tile([1, 1], bf16)
        smT = persist.tile([1, 1], fp32)

        def xcpad_tile():
            return scan_a.tile([128, NDT, NB, L], bf16, tag="dA", name="xcin")

        def dA_tile():
            return scan_a.tile([128, NDT, L], bf16, tag="dA", name="dA")

        def mov2_tile():
            return mov2.tile([128, NDT, TOK], bf16, tag="m2", name="m2t")

        # =============== fusion MLP ===============
        featsb = mov2.tile([128, NDT, TOK], bf16, tag="m2")
        nc.gpsimd.dma_start(out=featsb[:, :, :2 * N_EP], in_=d_featT[:])
        fW1 = wpool.tile([128, 4, 8, 128], bf16, tag="Wi")
        nc.gpsimd.dma_start(out=fW1[:, :, 0:4, :],
                            in_=d_fW1[:].rearrange("k p m f -> p k m f"))
        fb1 = small.tile([128, 4], fp32, tag="bias41")
        nc.gpsimd.dma_start(out=fb1, in_=d_fb1[:])
        fW2 = wpool.tile([128, 4, 4, 128], bf16, tag="Wo")
        nc.gpsimd.dma_start(out=fW2, in_=d_fW2[:].rearrange("k p m f -> p k m f"))
        fb2 = small.tile([128, 4], fp32, tag="bias42")
        nc.gpsimd.dma_start(out=fb2, in_=d_fb2[:])
        sum_sb = small.tile([128, 4, N_SUM], fp32, tag="sums")
        nc.gpsimd.dma_start(out=sum_sb, in_=d_sumT[:])
        tc.strict_bb_all_engine_barrier()

        h1 = scan_a.tile([128, NDT, TOK], bf16, tag="dA")
        CS = 510
        for mt in range(4):
            for chk in range(4):
                sl = slice(chk * CS, (chk + 1) * CS)
                pt = ppool.tile([128, 512], fp32, tag="mm")
                for kt in range(4):
                    nc.tensor.matmul(pt[:, :CS], fW1[:, kt, mt, :],
                                     featsb[:, kt, sl],
                                     start=(kt == 0), stop=(kt == 3))
                nc.scalar.activation(out=h1[:, mt, sl], in_=pt[:, :CS],
                                     func=AF.Gelu, bias=fb1[:, mt:mt + 1],
                                     scale=1.0)
        for mt in range(4):
            for chk in range(4):
                sl = slice(chk * CS, (chk + 1) * CS)
                pt = ppool.tile([128, 512], fp32, tag="mm")
                for kt in range(4):
                    nc.tensor.matmul(pt[:, :CS], fW2[:, kt, mt, :],
                                     h1[:, kt, sl],
                                     start=(kt == 0), stop=(kt == 3))
                for b in range(NB):
                    lo = max(chk * CS, b * N_EP)
                    hi = min((chk + 1) * CS, (b + 1) * N_EP)
                    if lo >= hi:
                        continue
                    xlo = b * L + N_SUM + (lo - b * N_EP)
                    nc.scalar.activation(
                        out=x[:, mt, xlo:xlo + hi - lo],
                        in_=pt[:, lo - chk * CS:hi - chk * CS],
                        func=AF.Identity, bias=fb2[:, mt:mt + 1], scale=1.0)
        for b in range(NB):
            nc.vector.tensor_copy(x[:, :, b * L:b * L + N_SUM], sum_sb)

        # =============== layer norm ===============
        def layer_norm(lrow, out16):
            xsq = scan_a.tile([128, NDT, TOK], bf16, tag="dA")
            for i in range(NDT):
                nc.scalar.activation(out=xsq[:, i, :], in_=x[:, i, :],
                                     func=AF.Square, bias=0.0, scale=1.0)
            for which, src, lhs, dst in ((0, x, ones_f, s1m),
                                         (1, xsq, ones_b, s2v)):
                for chk in range(4):
                    sl = slice(chk * 512, (chk + 1) * 512)
                    pt = ppool.tile([1, 512], fp32, tag="stat")
                    for i in range(NDT):
                        nc.tensor.matmul(pt, lhs, src[:, i, sl],
                                         start=(i == 0), stop=(i == NDT - 1))
                    nc.vector.tensor_copy(dst[:, sl], pt)
            mean = s1m
            nc.vector.tensor_scalar_mul(mean, s1m, 1.0 / D_MODEL)
            nc.vector.tensor_tensor(m2t, mean, mean, op=AL.mult)
            var = s2v
            nc.vector.scalar_tensor_tensor(var, s2v, 1.0 / D_MODEL, m2t,
                                           op0=AL.mult, op1=AL.subtract)
            nc.scalar.activation(out=var, in_=var, func=AF.Sqrt,
                                 bias=eps_sb, scale=1.0)
            nc.vector.reciprocal(var, var)
            gb_sb = small.tile([128, 8], fp32, tag="gb")
            nc.gpsimd.dma_start(out=gb_sb, in_=d_lngb[lrow])
            for b in range(NB):
                t0b = b * L
                stg = stgp.tile([1, 2, L], bf16, tag="stg")
                nc.sync.dma_start(out=stg[:, 0, :], in_=s1m[:, t0b:t0b + L])
                nc.sync.dma_start(out=stg[:, 1, :], in_=s2v[:, t0b:t0b + L])
                meanB = bcast.tile([128, L], bf16, tag="bc")
                nc.gpsimd.partition_broadcast(meanB, stg[:, 0, :])
                rstdB = bcast.tile([128, L], bf16, tag="bc")
                nc.gpsimd.partition_broadcast(rstdB, stg[:, 1, :])
                for i in range(NDT):
                    for chk in range(2):
                        sl = slice(t0b + chk * 512, t0b + (chk + 1) * 512)
                        sb_ = slice(chk * 512, (chk + 1) * 512)
                        t1 = small.tile([128, 512], bf16, tag="lnt")
                        nc.vector.tensor_tensor(t1, x[:, i, sl], meanB[:, sb_],
                                                op=AL.subtract)
                        nc.vector.tensor_tensor(t1, t1, rstdB[:, sb_], op=AL.mult)
                        nc.vector.tensor_scalar(out=out16[:, i, sl], in0=t1,
                                                scalar1=gb_sb[:, i:i + 1],
                                                scalar2=gb_sb[:, 4 + i:5 + i],
                                                op0=AL.mult, op1=AL.add)

        # =============== free-dim reversal (paired PE transposes) =========
        def reverse_into(src16, dst16, add_into_x=False):
            for i in range(NDT):
                for b in range(NB):
                    for tb in range(8):
                        s0 = b * L + tb * 128
                        dpos = b * L + (7 - tb) * 128
                        p1 = ppool.tile([128, 128], bf16, tag="rev")
                        nc.tensor.transpose(p1, src16[:, i, s0:s0 + 128], ident16)
                        t1 = small.tile([128, 128], bf16, tag="revs")
                        nc.scalar.copy(t1, p1)
                        p2 = ppool.tile([128, 128], bf16, tag="rev")
                        nc.tensor.transpose(p2, t1, J16)
                        if add_into_x:
                            nc.vector.tensor_tensor(
                                x[:, i, dpos:dpos + 128],
                                x[:, i, dpos:dpos + 128], p2, op=AL.add)
                        else:
                            nc.scalar.copy(dst16[:, i, dpos:dpos + 128], p2)

        # =============== layers ===============
        for l in range(N_LAYERS):
            layer_norm(l, hb16)
            for r in range(2):
                lr = 2 * l + r
                if r == 1:
                    hb = mov2_tile()
                    reverse_into(hb16, hb)
                else:
                    hb = hb16
                Wi = wpool.tile([128, 4, 8, 128], bf16, tag="Wi")
                nc.gpsimd.dma_start(out=Wi, in_=d_WiT[lr].rearrange(
                    "k p m f -> p k m f"))
                Wx = wpool.tile([128, 4, 2, 128], bf16, tag="Wx")
                nc.gpsimd.dma_start(out=Wx, in_=d_WxT[lr].rearrange(
                    "k p m f -> p k m f"))
                Wdt = wpool.tile([32, 4, 128], bf16, tag="Wdt")
                nc.gpsimd.dma_start(out=Wdt, in_=d_WdtT[lr])
                Wo = wpool.tile([128, 4, 4, 128], bf16, tag="Wo")
                nc.gpsimd.dma_start(out=Wo, in_=d_WoT[lr].rearrange(
                    "k p m f -> p k m f"))
                smw = small.tile([128, 4, 8], fp32, tag="smallw")
                nc.gpsimd.dma_start(out=smw, in_=d_smallw[lr])
                if not A_uniform:
                    An_sb = small.tile([128, 4, D_STATE], fp32, tag="An")
                    nc.gpsimd.dma_start(out=An_sb, in_=d_Aneg[lr])

                # ---- in_proj ----
                xcpad = xcpad_tile()
                xcpf = xcpad.rearrange("p dt b t -> p (dt b t)")
                for mt in range(8):
                    for chk in range(4):
                        sl = slice(chk * 512, (chk + 1) * 512)
                        pt = ppool.tile([128, 512], fp32, tag="mm")
                        for kt in range(4):
                            nc.tensor.matmul(pt, Wi[:, kt, mt, :], hb[:, kt, sl],
                                             start=(kt == 0), stop=(kt == 3))
                        if mt < 4:
                            b = chk // 2
                            off = mt * NB * L + b * L + (chk % 2) * 512
                            nc.scalar.copy(xcpf[:, off:off + 512], pt)
                        else:
                            nc.scalar.copy(zg[:, mt - 4, sl], pt)

                # ---- conv + silu (in-place accumulate in xc, bf16) ----
                for i in range(NDT):
                    for b in range(NB):
                        xsl = xc[:, i, b * L + PAD:(b + 1) * L]
                        xin = xcpad[:, i, b, :]
                        nc.vector.tensor_scalar(
                            out=xsl, in0=xin[:, 0:L - PAD],
                            scalar1=smw[:, i, 0:1], scalar2=None, op0=AL.mult)
                        for k in range(1, D_CONV):
                            nc.vector.scalar_tensor_tensor(
                                xsl, xin[:, k:L - PAD + k],
                                smw[:, i, k:k + 1], xsl, op0=AL.mult, op1=AL.add)
                        # edge columns t = 0..2 (causal zero-pad)
                        for t in range(PAD):
                            esl = xc[:, i, b * L + t:b * L + t + 1]
                            nc.vector.tensor_scalar(
                                out=esl, in0=xin[:, 0:1],
                                scalar1=smw[:, i, PAD - t:PAD - t + 1],
                                scalar2=None, op0=AL.mult)
                            for k in range(1, t + 1):
                                nc.vector.scalar_tensor_tensor(
                                    esl, xin[:, k:k + 1],
                                    smw[:, i, PAD - t + k:PAD - t + k + 1], esl,
                                    op0=AL.mult, op1=AL.add)
                for i in range(NDT):
                    nc.scalar.activation(out=xc[:, i, :], in_=xc[:, i, :],
                                         func=AF.Silu, bias=smw[:, i, 4:5],
                                         scale=1.0)

                # ---- x_proj ----
                for mt in range(2):
                    for chk in range(4):
                        sl = slice(chk * 512, (chk + 1) * 512)
                        pt = ppool.tile([128, 512], fp32, tag="mm")
                        for kt in range(4):
                            nc.tensor.matmul(pt, Wx[:, kt, mt, :], xc[:, kt, sl],
                                             start=(kt == 0), stop=(kt == 3))
                        if mt == 0:
                            nc.vector.tensor_copy(proj[:, sl], pt)
                        else:
                            nc.vector.tensor_copy(cmhi[:, sl], pt[0:32, :])

                # ---- dt = softplus ----
                for mt in range(4):
                    for chk in range(4):
                        sl = slice(chk * 512, (chk + 1) * 512)
                        pt = ppool.tile([128, 512], fp32, tag="mm")
                        nc.tensor.matmul(pt, Wdt[:, mt, :], proj[0:32, sl],
                                         start=True, stop=True)
                        et = small.tile([128, 512], bf16, tag="lnt")
                        nc.scalar.activation(out=et, in_=pt, func=AF.Exp,
                                             bias=smw[:, mt, 5:6], scale=1.0)
                        nc.scalar.activation(out=dtt[:, mt, sl], in_=et,
                                             func=AF.Ln, bias=ones_f, scale=1.0)
                for i in range(NDT):
                    nc.vector.tensor_tensor(ut[:, i, :], dtt[:, i, :],
                                            xc[:, i, :], op=AL.mult)

                # ---- selective scan (per sample b, per channel n) ----
                y16 = mov2.tile([128, NDT, TOK], bf16, tag="m2", name="y16")
                for b in range(NB):
                    tb0 = b * L
                    for n in range(D_STATE):
                        dA = dA_tile()
                        dAb = dA
                        if A_uniform:
                            nc.scalar.activation(
                                out=dAb, in_=dtt[:, :, tb0:tb0 + L],
                                func=AF.Exp, bias=0.0, scale=float(A_vals[lr, n]))
                        else:
                            for i in range(NDT):
                                nc.scalar.activation(
                                    out=dAb[:, i, :],
                                    in_=dtt[:, i, tb0:tb0 + L], func=AF.Exp,
                                    bias=0.0, scale=An_sb[:, i, n:n + 1])
                        nc.vector.memset(dAb[:, :, 0:1], 0.0)
                        stgbc = stgp.tile([1, 2, L], bf16, tag="stg")
                        nc.sync.dma_start(out=stgbc[:, 0, :],
                                          in_=proj[32 + n:33 + n, tb0:tb0 + L])
                        if n < 32:
                            nc.sync.dma_start(
                                out=stgbc[:, 1, :],
                                in_=proj[96 + n:97 + n, tb0:tb0 + L])
                        else:
                            nc.sync.dma_start(
                                out=stgbc[:, 1, :],
                                in_=cmhi[n - 32:n - 31, tb0:tb0 + L])
                        bB = bcast.tile([128, L], bf16, tag="bc")
                        nc.gpsimd.partition_broadcast(bB, stgbc[:, 0, :])
                        bBr = bass.AP(tensor=bB.tensor, offset=bB.offset,
                                        ap=[list(bB.ap[0]), [0, NDT], [1, L]])
                        uB = uB2.tile([128, NDT, L], bf16, tag="uBh2")
                        nc.vector.tensor_tensor(uB, ut[:, :, tb0:tb0 + L], bBr,
                                                op=AL.mult)
                        uBf = uB.rearrange("p dt t -> p (dt t)")
                        dAf = dA.rearrange("p dt t -> p (dt t)")
                        nc.vector.tensor_tensor_scan(uBf, dAf, uBf, 0.0,
                                                     op0=AL.mult, op1=AL.add)
                        bC = bcast.tile([128, L], bf16, tag="bc")
                        nc.gpsimd.partition_broadcast(bC, stgbc[:, 1, :])
                        bCr = bass.AP(tensor=bC.tensor, offset=bC.offset,
                                        ap=[list(bC.ap[0]), [0, NDT], [1, L]])
                        nc.vector.tensor_tensor(uB, uB, bCr, op=AL.mult)
                        ysl = y16[:, :, tb0:tb0 + L]
                        if n == 0:
                            nc.vector.tensor_copy(ysl, uB)
                        else:
                            nc.vector.tensor_tensor(ysl, ysl, uB, op=AL.add)
                    # y2 = D_skip*xc + y ; then *= silu(z)
                    for i in range(NDT):
                        sl = slice(tb0, tb0 + L)
                        nc.vector.scalar_tensor_tensor(
                            ut[:, i, sl], xc[:, i, sl], smw[:, i, 6:7],
                            y16[:, i, sl], op0=AL.mult, op1=AL.add)

                # gating: reuse ut as y2 storage
                for i in range(NDT):
                    nc.scalar.activation(out=zg[:, i, :], in_=zg[:, i, :],
                                         func=AF.Silu, bias=0.0, scale=1.0)
                    nc.vector.tensor_tensor(ut[:, i, :], ut[:, i, :],
                                            zg[:, i, :], op=AL.mult)

                # ---- out_proj + residual ----
                if r == 0:
                    for mt in range(4):
                        for chk in range(4):
                            sl = slice(chk * 512, (chk + 1) * 512)
                            pt = ppool.tile([128, 512], fp32, tag="mm")
                            for kt in range(4):
                                nc.tensor.matmul(pt, Wo[:, kt, mt, :],
                                                 ut[:, kt, sl],
                                                 start=(kt == 0), stop=(kt == 3))
                            nc.vector.tensor_tensor(x[:, mt, sl], x[:, mt, sl],
                                                    pt, op=AL.add)
                else:
                    dxs = mov2_tile()
                    for mt in range(4):
                        for chk in range(4):
                            sl = slice(chk * 512, (chk + 1) * 512)
                            pt = ppool.tile([128, 512], fp32, tag="mm")
                            for kt in range(4):
                                nc.tensor.matmul(pt, Wo[:, kt, mt, :],
                                                 ut[:, kt, sl],
                                                 start=(kt == 0), stop=(kt == 3))
                            nc.scalar.copy(dxs[:, mt, sl], pt)
                    reverse_into(dxs, None, add_into_x=True)

        # =============== final norm + outputs ===============
        layer_norm(12, hb16)
        wlog = wlogt
        for chk in range(4):
            sl = slice(chk * 512, (chk + 1) * 512)
            pt = ppool.tile([1, 512], fp32, tag="stat")
            for kt in range(4):
                nc.tensor.matmul(pt, attnW_sb[:, kt:kt + 1], hb16[:, kt, sl],
                                 start=(kt == 0), stop=(kt == 3))
            nc.vector.tensor_copy(wlog[:, sl], pt)
        nc.vector.tensor_scalar_add(wlog, wlog, float(attnb))
        wsm = wsmt
        for b in range(NB):
            sl = slice(b * L, (b + 1) * L)
            mx = mxT
            nc.vector.tensor_reduce(mx, wlog[:, sl], axis=mybir.AxisListType.X,
                                    op=AL.max)
            nc.vector.tensor_scalar_mul(mx, mx, -1.0)
            nc.scalar.activation(out=wsm[:, sl], in_=wlog[:, sl], func=AF.Exp,
                                 bias=mx, scale=1.0)
            sm = smT
            nc.vector.tensor_reduce(sm, wsm[:, sl], axis=mybir.AxisListType.X,
                                    op=AL.add)
            nc.vector.reciprocal(sm, sm)
            nc.vector.tensor_scalar(out=wsm[:, sl], in0=wsm[:, sl],
                                    scalar1=sm, scalar2=None, op0=AL.mult)
        dayt = scan_a.tile([128, NDT, TOK], bf16, tag="dA")
        for b in range(NB):
            t0b = b * L
            stgw = stgp.tile([1, 2, L], bf16, tag="stg")
            nc.sync.dma_start(out=stgw[:, 0, :], in_=wsmt[:, t0b:t0b + L])
            wbc = bcast.tile([128, L], bf16, tag="bc")
            nc.gpsimd.partition_broadcast(wbc, stgw[:, 0, :])
            for i in range(NDT):
                nc.vector.tensor_tensor(dayt[:, i, t0b:t0b + L],
                                        hb16[:, i, t0b:t0b + L], wbc, op=AL.mult)
        dayr = small.tile([128, NDT, NB], fp32, tag="sums")
        nc.vector.tensor_reduce(
            dayr, dayt.rearrange("p dt (b t) -> p dt b t", b=NB),
            axis=mybir.AxisListType.X, op=AL.add)
        for b in range(NB):
            nc.sync.dma_start(
                out=o_day[b].rearrange("(dt p) -> p dt", p=128),
                in_=dayr[:, :, b])
        for b in range(NB):
            for tb in range(8):
                tlo = b * L + N_SUM + tb * 128
                ncols = min(128, (b + 1) * L - tlo)
                ctxT = small.tile([128, D_MODEL], fp32, tag="ctxT")
                for i in range(NDT):
                    p1 = ppool.tile([128, 128], bf16, tag="rev")
                    nc.tensor.transpose(p1[:ncols, :],
                                        hb16[:, i, tlo:tlo + ncols], ident16)
                    nc.scalar.copy(ctxT[:ncols, i * 128:(i + 1) * 128],
                                   p1[:ncols, :])
                nc.sync.dma_start(
                    out=o_ctx[b, tb * 128:tb * 128 + ncols, :],
                    in_=ctxT[:ncols, :])

    nc.compile()
    return nc


def kernel(**inputs):
    shared, per_core = _prep_host(inputs)
    if "prog" not in _BUILD_CACHE:
        _BUILD_CACHE["prog"] = _build_program(
            shared["_A_vals"], shared["_A_uniform"], shared["_attnb"])
    nc = _BUILD_CACHE["prog"]
    from concourse.bass_utils import run_bass_kernel_spmd
    shared_arrs = {k: v for k, v in shared.items() if not k.startswith("_")}
    in_maps = []
    for c in range(NCORES):
        m = dict(shared_arrs)
        m.update(per_core[c])
        in_maps.append(m)
    res = run_bass_kernel_spmd(nc, in_maps, list(range(NCORES)))
    day = np.concatenate([res.results[c]["day"] for c in range(NCORES)], 0)
    ctx = np.concatenate([res.results[c]["ctx"] for c in range(NCORES)], 0)
    return day.astype(np.float32), ctx.astype(np.float32)
